# revision 25
# baseline (speedup 1.0000x reference)
"""Single-head causal attention (B=4, T=2048, C=1024, fp32) on 8 Trainium2 cores.

Sharding: core = (batch b = core//2, half h = core%2). Each core computes the
attention output for 1024 query rows of one batch: 4 q-blocks of 256 rows,
assigned so every core has an identical 4-slot [16, 12, 8, 4] causal unit
schedule (40 column-tile units of 128 keys each; 36 real + 4 dummies).
  h=0 -> g = [7, 4, 3, 0],  h=1 -> g = [6, 5, 2, 1]
Causality and dummy suppression use additive masks applied ONLY to the last
active slot of each k-window j (uniform rule: the masked slot is 3 - j//4);
all earlier active slots are fully below the diagonal by construction.

K/V projections are deduplicated across the core pair: each core projects its
OWN half of the sequence (1024 rows, host-sliced x input), stages K/V halves
to a DRAM bounce buffer, and a pairwise AllGather ([[0,1],[2,3],[4,5],[6,7]])
exchanges them while the Q projection runs; full K/V are then DMA'd back into
SBUF in natural k order (rank r of the gather = sequence half r).

Everything runs in bf16 (inputs host-cast; fp32 PSUM accumulation), which
matches fp32r PE throughput, halves DMA/SBUF traffic, and keeps rel err
~5e-3 << 2e-2. Attention: S k-major (one PSUM tile per k-window spanning all
active slots, 512-wide matmuls), exp into a persistent PM buffer, AV
slot-major. Softmax needs no running max (|S|*scale < ~4):
out = sum_j exp(S_j) V_j / sum_j exp(S_j).
"""

import os
import sys

import numpy as np

for _p in ("/opt/trn_rl_repo", os.path.expanduser("~/.axon_site/_ro/trn_rl_repo")):
    if os.path.isdir(_p) and _p not in sys.path:
        sys.path.insert(0, _p)

B, T, C = 4, 2048, 1024
QBLK = 256
NSLOT = 4
SLOT_UNITS = [16, 12, 8, 4]
SLOT_OFF = [0, 16, 28, 36]
NUNITS = sum(SLOT_UNITS)          # 40
M_OF_J = [4, 4, 4, 4, 3, 3, 3, 3, 2, 2, 2, 2, 1, 1, 1, 1]
ASSIGN = {0: [7, 4, 3, 0], 1: [6, 5, 2, 1]}
SCALE = float(C) ** -0.5
GROUPS = [[0, 1], [2, 3], [4, 5], [6, 7]]

_CACHE = {}


def _build_nc(reps=1, dedup_kv=True, fake_cc=False):
    """fake_cc=True replaces the AllGathers with local DMA copies of the same
    size — WRONG results, used only to simulate the collective-free timeline
    (TimelineSim's collective cost model is a pessimistic inter-chip one)."""
    import concourse.tile as tile
    from concourse import bacc, mybir
    from contextlib import ExitStack

    f32 = mybir.dt.float32
    bf16 = mybir.dt.bfloat16
    Exp = mybir.ActivationFunctionType.Exp
    Copy = mybir.ActivationFunctionType.Copy

    nc = bacc.Bacc("TRN2", target_bir_lowering=False, debug=False)

    TH = T // 2 if dedup_kv else T            # own k rows projected per core
    NW = TH // 512                            # 512-wide x windows (2 or 4)
    xhT = nc.dram_tensor("xhT", [C, TH], bf16, kind="ExternalInput").ap()
    xqT = nc.dram_tensor("xqT", [C, 1024], bf16, kind="ExternalInput").ap()
    wkT = nc.dram_tensor("wkT", [C, C], bf16, kind="ExternalInput").ap()
    wqT = nc.dram_tensor("wqT", [C, C], bf16, kind="ExternalInput").ap()
    wvT = nc.dram_tensor("wvT", [C, C], bf16, kind="ExternalInput").ap()
    masks = nc.dram_tensor("masks", [16, 128, QBLK], bf16, kind="ExternalInput").ap()
    out = nc.dram_tensor("out", [1024, C], bf16, kind="ExternalOutput").ap()

    def mm(ps, lhsT, rhs, s0, s1):
        nc.tensor.matmul(ps, lhsT, rhs, start=s0, stop=s1)

    def load_chunked(pool, name, dram_ap, cols, tag=None):
        """[1024, cols] DRAM slice -> [128, 8*cols] tile (cin-chunk ci at
        free offset ci*cols)."""
        t = pool.tile([128, 8 * cols], bf16, tag=tag or name, name=name)
        nc.sync.dma_start(
            out=t[:].rearrange("p (a m) -> p a m", a=8),
            in_=dram_ap.rearrange("(a p) m -> p a m", p=128),
        )
        return t

    with tile.TileContext(nc) as tc, ExitStack() as ctx:
        persist = ctx.enter_context(tc.tile_pool(name="persist", bufs=1))
        misc = ctx.enter_context(tc.tile_pool(name="misc", bufs=1))

        KT = [persist.tile([128, T], bf16, tag=f"kt{i}", name=f"kt{i}") for i in range(8)]
        V = [persist.tile([128, C], bf16, tag=f"v{i}", name=f"v{i}") for i in range(16)]
        QT = [persist.tile([128, 1024], bf16, tag=f"qt{i}", name=f"qt{i}") for i in range(8)]
        maskt = persist.tile([128, 16 * QBLK], bf16, tag="maskt", name="maskt")

        ones_f = misc.tile([128, 2], f32, name="ones_f")
        nc.vector.memset(ones_f[:], 1.0)
        ones = misc.tile([128, 2], bf16, name="ones")
        nc.scalar.copy(ones[:], ones_f[:])
        # warm the ACT function table (Exp) during the initial DMA stall
        wrm = misc.tile([128, 2], f32, name="wrm")
        nc.scalar.activation(wrm[:], ones_f[:], Exp, scale=1.0)

        for rep in range(reps):
            r = f"r{rep}_"
            pp_ctx = tc.psum_pool(name="pp", bufs=8)
            pp_pool = pp_ctx.__enter__()
            # ============ projection scope ===============================
            # Order: K-own (gather-K fires ~28us in), V-own (gather-V at
            # ~56us), Q — so each gather hides behind remaining projections.
            with tc.tile_pool(name="xp", bufs=1) as xpool, \
                 tc.tile_pool(name="wp", bufs=1) as wpool, \
                 tc.tile_pool(name="stg", bufs=1) as stg, \
                 tc.tile_pool(name="dram", bufs=1, space="DRAM") as dram:
                if dedup_kv:
                    k_in = dram.tile([8 * 128, 1024], bf16, tag="ki", name=f"{r}ki")
                    k_out = dram.tile([16 * 128, 1024], bf16, tag="ko", name=f"{r}ko")
                    v_in = dram.tile([8 * 128, 1024], bf16, tag="vi", name=f"{r}vi")
                    v_out = dram.tile([16 * 128, 1024], bf16, tag="vo", name=f"{r}vo")
                # DMA issue order = priority order
                xw0a = load_chunked(xpool, f"{r}xw0a", xhT[:, 0:256], 256, tag="xw0a")
                wk0ab = load_chunked(wpool, f"{r}wk0ab", wkT[:, 0:256], 256, tag="wk0ab")
                wk = [load_chunked(wpool, f"{r}wk{i}", wkT[:, 512 * i:512 * (i + 1)],
                                   512, tag=f"wk{i}") for i in range(2)]
                xw = [load_chunked(xpool, f"{r}xw0", xhT[:, 0:512], 512, tag="xw0")]
                for w in range(1, NW):
                    xw.append(load_chunked(xpool, f"{r}xw{w}",
                                           xhT[:, 512 * w:512 * (w + 1)], 512,
                                           tag=f"xw{w}"))
                wv = [load_chunked(wpool, f"{r}wv{i}", wvT[:, 512 * i:512 * (i + 1)],
                                   512, tag=f"wv{i}") for i in range(2)]
                wq = [load_chunked(wpool, f"{r}wq{i}", wqT[:, 512 * i:512 * (i + 1)],
                                   512, tag=f"wq{i}") for i in range(2)]
                xq = load_chunked(wpool, f"{r}xq", xqT, 1024, tag="xq")
                nc.sync.dma_start(
                    out=maskt[:].rearrange("p (u m) -> p u m", u=16),
                    in_=masks.rearrange("u p m -> p u m"),
                )

                # ---- phase K: own-half KT ([co, k] bf16), w-outer -------
                KTo = ([stg.tile([128, TH], bf16, tag=f"ks{c}", name=f"{r}ks{c}")
                        for c in range(8)] if dedup_kv else KT)
                pk = pv = pq = pp_pool
                if True:
                    # window 0 in two 256-wide half-passes: the first needs
                    # only xw0a (0.5 MB) + wk (2 MB) before compute starts
                    for q2 in range(2):
                        for h in range(2):
                            for co4 in range(4):
                                c = 4 * h + co4
                                ps = pk.tile([128, 512], f32, tag="pp", name=f"{r}k0_{c}_{q2}")
                                src_t = xw0a if q2 == 0 else xw[0]
                                off = 0 if q2 == 0 else 256
                                cw = 256 if q2 == 0 else 512
                                for ci in range(8):
                                    lhsT = (wk0ab[:, 256 * ci + 128 * co4:
                                                  256 * ci + 128 * (co4 + 1)]
                                            if (q2 == 0 and h == 0 and co4 < 2) else
                                            wk[h][:, 512 * ci + 128 * co4:
                                                  512 * ci + 128 * (co4 + 1)])
                                    mm(ps[:, 0:256], lhsT,
                                       src_t[:, cw * ci + off: cw * ci + off + 256],
                                       ci == 0, ci == 7)
                                nc.scalar.copy(KTo[c][:, 256 * q2:256 * (q2 + 1)], ps[:, 0:256])
                    for w in range(1, NW):
                        for h in range(2):
                            for co4 in range(4):
                                c = 4 * h + co4
                                ps = pk.tile([128, 512], f32, tag="pp", name=f"{r}k{c}_{w}")
                                for ci in range(8):
                                    mm(ps[:],
                                       wk[h][:, 512 * ci + 128 * co4: 512 * ci + 128 * (co4 + 1)],
                                       xw[w][:, 512 * ci:512 * (ci + 1)],
                                       ci == 0, ci == 7)
                                nc.scalar.copy(KTo[c][:, 512 * w:512 * (w + 1)], ps[:])
                                if dedup_kv and w == NW - 1:
                                    nc.sync.dma_start(
                                        out=k_in[128 * c:128 * (c + 1), :],
                                        in_=KTo[c][:],
                                    )
                if dedup_kv:
                    if fake_cc:
                        nc.sync.dma_start(out=k_out[0:1024, :], in_=k_in[:])
                        nc.sync.dma_start(out=k_out[1024:2048, :], in_=k_in[:])
                    else:
                        nc.gpsimd.collective_compute(
                            "AllGather", mybir.AluOpType.bypass, replica_groups=GROUPS,
                            ins=[k_in.opt()], outs=[k_out.opt()],
                        )
                # ---- phase V: own-half V ([k, co] bf16) -----------------
                if True:
                    for w in range(NW):
                        for kc4 in range(4):
                            kc = 4 * w + kc4
                            dst = (stg.tile([128, C], bf16, tag="vstg", bufs=3,
                                            name=f"{r}vs{kc}")
                                   if dedup_kv else V[kc])
                            pss = [pv.tile([128, 512], f32, tag="pp",
                                           name=f"{r}v{kc}_{half}")
                                   for half in range(2)]
                            for ci in range(8):
                                for half in range(2):
                                    mm(pss[half][:],
                                       xw[w][:, 512 * ci + 128 * kc4: 512 * ci + 128 * (kc4 + 1)],
                                       wv[half][:, 512 * ci:512 * (ci + 1)],
                                       ci == 0, ci == 7)
                            for half in range(2):
                                nc.scalar.copy(dst[:, 512 * half:512 * (half + 1)],
                                               pss[half][:])
                            if dedup_kv:
                                nc.sync.dma_start(
                                    out=v_in[128 * kc:128 * (kc + 1), :], in_=dst[:],
                                )
                if dedup_kv:
                    if fake_cc:
                        nc.sync.dma_start(out=v_out[0:1024, :], in_=v_in[:])
                        nc.sync.dma_start(out=v_out[1024:2048, :], in_=v_in[:])
                    else:
                        nc.gpsimd.collective_compute(
                            "AllGather", mybir.AluOpType.bypass, replica_groups=GROUPS,
                            ins=[v_in.opt()], outs=[v_out.opt()],
                        )
                    # K readback emitted after V staging so its SP-queue wait on
                    # gather-K does not delay the V staging dma_starts
                    for rank in range(2):
                        for c in range(8):
                            nc.sync.dma_start(
                                out=KT[c][:, 1024 * rank:1024 * (rank + 1)],
                                in_=k_out[1024 * rank + 128 * c:
                                          1024 * rank + 128 * (c + 1), :],
                            )
                # ---- phase Q: QT[c] = wq_chunk^T @ xq ([co, q] bf16) ----
                if True:
                    for h in range(2):
                        for co4 in range(4):
                            c = 4 * h + co4
                            pss = [pq.tile([128, 512], f32, tag="pp",
                                           name=f"{r}q{c}_{qw}") for qw in range(2)]
                            for ci in range(8):
                                for qw in range(2):
                                    mm(pss[qw][:],
                                       wq[h][:, 512 * ci + 128 * co4: 512 * ci + 128 * (co4 + 1)],
                                       xq[:, 1024 * ci + 512 * qw: 1024 * ci + 512 * (qw + 1)],
                                       ci == 0, ci == 7)
                            for qw in range(2):
                                nc.scalar.copy(QT[c][:, 512 * qw:512 * (qw + 1)], pss[qw][:])

                # ---- V readback emitted after Q so its SP-queue work does
                # not delay the Q psum-pool barrier --------------------------
                if dedup_kv:
                    for kc in range(16):
                        rank, l = kc // 8, kc % 8
                        nc.sync.dma_start(
                            out=V[kc][:],
                            in_=v_out[1024 * rank + 128 * l:
                                      1024 * rank + 128 * (l + 1), :],
                        )

            # ============ attention scope ================================
            with tc.tile_pool(name="pmp", bufs=1) as pmp, \
                 tc.tile_pool(name="smp", bufs=2) as smp, \
                 tc.tile_pool(name="outp", bufs=2) as outp, \
                 tc.tile_pool(name="linvp", bufs=2) as linvp:
                PM = pmp.tile([128, NUNITS * QBLK], bf16, tag="pm", name=f"{r}pm")

                # ---- S phase, k-major: SJ[j] spans all active slots, as
                # 1-bank halves from the shared psum ring (no pool barrier) --
                if True:
                    for j in range(16):
                        m = M_OF_J[j]
                        w = m * QBLK
                        SJ = [pp_pool.tile([128, 512], f32, tag="pp",
                                           name=f"{r}sj{j}_{i}")
                              for i in range((w + 511) // 512)]

                        def sj_slice(c0, c1):
                            t = SJ[c0 // 512]
                            return t[:, c0 % 512:(c0 % 512) + (c1 - c0)]

                        for ci in range(8):
                            for c0 in range(0, w, 512):
                                c1 = min(c0 + 512, w)
                                mm(sj_slice(c0, c1),
                                   KT[ci][:, 128 * j:128 * (j + 1)],
                                   QT[ci][:, c0:c1],
                                   ci == 0, ci == 7)
                        sm = smp.tile([128, QBLK], bf16, tag="sm", name=f"{r}sm{j}")
                        nc.vector.tensor_add(sm[:], sj_slice((m - 1) * QBLK, m * QBLK),
                                             maskt[:, QBLK * j:QBLK * (j + 1)])
                        for s in range(m - 1):
                            o = (SLOT_OFF[s] + j) * QBLK
                            nc.scalar.activation(PM[:, o:o + QBLK],
                                                 sj_slice(QBLK * s, QBLK * (s + 1)),
                                                 Exp, scale=SCALE)
                        o = (SLOT_OFF[m - 1] + j) * QBLK
                        nc.scalar.activation(PM[:, o:o + QBLK], sm[:], Exp, scale=SCALE)

                # ---- AV phase, slot-major ------------------------------
                if True:
                    for s in range(NSLOT):
                        n = SLOT_UNITS[s]
                        o_ps = [[pp_pool.tile([128, 512], f32, tag="pp",
                                              name=f"{r}o{qc}_{ch}_{s}")
                                 for ch in range(2)] for qc in range(2)]
                        l_all = pp_pool.tile([128, 512], f32, tag="pp", name=f"{r}l_{s}")
                        l_ps = [l_all[:, 2 * qc:2 * (qc + 1)] for qc in range(2)]
                        for qc in range(2):
                            for j in range(n):
                                base = (SLOT_OFF[s] + j) * QBLK
                                first, last = (j == 0), (j == n - 1)
                                pmc = PM[:, base + 128 * qc: base + 128 * (qc + 1)]
                                mm(o_ps[qc][0][:], pmc, V[j][:, 0:512], first, last)
                                mm(o_ps[qc][1][:], pmc, V[j][:, 512:1024], first, last)
                                mm(l_ps[qc], pmc, ones[:], first, last)
                            linv = linvp.tile([128, 1], f32, tag="linv", name=f"{r}li{s}_{qc}")
                            nc.vector.reciprocal(linv[:], l_ps[qc][:, 0:1].opt())
                            for ch in range(2):
                                ob = outp.tile([128, 512], bf16, tag=f"ob{ch}",
                                               name=f"{r}ob{s}_{qc}_{ch}")
                                nc.scalar.activation(ob[:], o_ps[qc][ch][:], Copy,
                                                     scale=linv[:])
                                nc.sync.dma_start(
                                    out=out[QBLK * s + 128 * qc: QBLK * s + 128 * (qc + 1),
                                            512 * ch:512 * (ch + 1)],
                                    in_=ob[:],
                                )
            pp_ctx.__exit__(None, None, None)
    nc.finalize()
    return nc


def _masks16(h):
    import ml_dtypes
    m = np.zeros((16, 128, QBLK), np.float32)
    p = np.arange(128)[:, None]
    q = np.arange(QBLK)[None, :]
    for j in range(16):
        s_star = 3 - j // 4
        g = ASSIGN[h][s_star]
        if j < 2 * g:
            pass                                   # fully below diagonal
        elif j == 2 * g:
            m[j] = np.where(p > q, -30000.0, 0.0)
        elif j == 2 * g + 1:
            m[j] = np.where(128 + p > q, -30000.0, 0.0)
        else:
            m[j] = -30000.0                        # dummy unit
    return m.astype(ml_dtypes.bfloat16)


def _get_built(reps=1):
    key = f"nc{reps}"
    if key not in _CACHE:
        _CACHE[key] = _build_nc(reps)
    if "masks" not in _CACHE:
        _CACHE["masks"] = {h: _masks16(h) for h in (0, 1)}
    return _CACHE[key], _CACHE["masks"]


def _in_maps(x, Wk, Wq, Wv, mks):
    import ml_dtypes
    bf = ml_dtypes.bfloat16
    xb = np.asarray(x, np.float32)
    wkT = np.ascontiguousarray(np.asarray(Wk, np.float32).T.astype(bf))
    wqT = np.ascontiguousarray(np.asarray(Wq, np.float32).T.astype(bf))
    wvT = np.ascontiguousarray(np.asarray(Wv, np.float32).T.astype(bf))

    in_maps = []
    for core in range(8):
        b, h = core // 2, core % 2
        xT_b = np.ascontiguousarray(xb[b].T.astype(bf))
        gs = ASSIGN[h]
        xqT = np.ascontiguousarray(
            np.concatenate([xT_b[:, 256 * g:256 * (g + 1)] for g in gs], axis=1)
        )
        xhT = np.ascontiguousarray(xT_b[:, 1024 * h:1024 * (h + 1)])
        in_maps.append({
            "xhT": xhT, "xqT": xqT,
            "wkT": wkT, "wqT": wqT, "wvT": wvT,
            "masks": mks[h],
        })
    return in_maps


def kernel(x, Wk, Wq, Wv, **_ignored):
    from concourse.bass_utils import run_bass_kernel_spmd

    nc, mks = _get_built(reps=1)
    in_maps = _in_maps(x, Wk, Wq, Wv, mks)
    res = run_bass_kernel_spmd(nc, in_maps, core_ids=list(range(8)))
    _CACHE["last_res"] = res

    out = np.empty((B, T, C), np.float32)
    for core in range(8):
        b, h = core // 2, core % 2
        o = np.asarray(res.results[core]["out"], np.float32)
        for s, g in enumerate(ASSIGN[h]):
            out[b, 256 * g:256 * (g + 1), :] = o[256 * s:256 * (s + 1), :]
    return out
